# revision 50
# baseline (speedup 1.0000x reference)
"""Multihead attention (B=2, S=2048, E=1024, H=16) on 8 TRN2 cores.

Sharding: tensor-parallel over heads — core c computes heads {2c, 2c+1}
(DOUT = 128 columns of the QKV projections) for the full sequence, then its
partial contribution to the output projection; the host sums the 8 partials
and adds the output bias.

v2 layout (per core), all 2-byte tensors in fp16 (same PE rate as bf16,
~8x lower rounding error):
  x^T [E, B*S] fp16 streamed as [128, 2048] tiles (4KB DMA lines, full HBM
  rate).  QKV projections produce Q^T/K^T/V^T [128, 4096] fp16 in SBUF.
  Attention per (batch, head) computes scores^T [kpos, q] tiles (lhsT = K^T
  slice, rhs = Q^T slice), exponentiates on ACT over [128, 1024] PSUM tiles,
  and multiplies by V via matmul with lhsT = [V | ones] so the softmax
  denominator falls out of the same accumulation (PSUM row 64).  The
  reciprocal runs on ACT (DVE reciprocal on a 1-partition tile is ~4us),
  is broadcast over 64 partitions by a tiny PE matmul, and the normalized
  context lands in ctxT [128, 4096] fp16.  The output projection contracts
  the 128 local head dims in fp16 and streams out partial sums as fp16.

Emission is software-pipelined so the PE never stalls on ACT: per t-pair
step the attnV matmuls of the PREVIOUS step are emitted after the scores
of the current one, and projection / out-projection chunks of the other
batch are interleaved between attention steps as PE gap filler.
"""

import numpy as np

# Problem constants (hardcoded per the task contract).
B, S, E, H = 2, 2048, 1024, 16
D = E // H          # 64
NSEQ = B * S        # 4096
NCORES = 8
DOUT = E // NCORES  # 128 = 2 heads x 64
KE = E // 128       # 8 contraction tiles over E
SEQT = 512          # q-block for attention
QB = S // SEQT      # 4 q-blocks per batch
KT = S // 128       # 16 kpos tiles per batch
ISD = float(D) ** -0.5

_PROGRAM = None


# ---------------------------------------------------------------------------
# Workarounds for this walrus build: at most ONE sync wait per instruction is
# reliably accepted ("Too many sync wait commands").  (1) tile's final drain
# gets one wait per logical proc — split them over single-wait SP NOPs;
# (2) a general post-pass moves any instruction's excess waits onto
# preceding same-engine NOPs (engine program order preserves semantics).
# ---------------------------------------------------------------------------


def _install_tile_drain_patch():
    import concourse.mybir as mybir
    import concourse.tile as tile
    from concourse.tile import ScopedClock

    if getattr(tile.TileContext, "_drain_patch_installed", False):
        return

    def _patched_drain_and_barrier(self, tick_clock, wait_clock):
        nc = self.nc
        carrier = nc.sync.nop(nofuse=True)
        wait_clock.add_sem_waits(
            carrier.ins, ScopedClock({None: tick_clock.global_clock})
        )
        si = carrier.ins.sync_info
        waits = list(si.on_wait) if si and si.on_wait else []
        ups = list(si.on_update) if si and si.on_update else []
        if len(waits) > 1:
            carrier.ins.sync_info = mybir.SyncInfo(on_wait=[waits[0]], on_update=ups)
            for w in waits[1:]:
                n2 = nc.sync.nop(nofuse=True)
                n2.ins.sync_info = mybir.SyncInfo(on_wait=[w], on_update=[])
        nc.sync.drain()
        nc.all_engine_barrier()
        popped = nc._tile_sem_poison_stack.pop()
        assert popped is self._sem_poison
        nc.clear_and_free_semaphores(list(self.sems.allocated().values()))
        nc.all_engine_barrier()

    tile.TileContext._drain_and_barrier = _patched_drain_and_barrier
    tile.TileContext._drain_patch_installed = True


MAX_WAITS = 1


def _split_excess_waits(nc):
    import concourse.mybir as mybir

    for bb in nc.main_func.blocks:
        il = list(bb.instructions)
        out = []
        changed = False
        for ins in il:
            si = ins.sync_info
            waits = list(si.on_wait) if si and si.on_wait else []
            if len(waits) > MAX_WAITS:
                changed = True
                extras = waits[: len(waits) - MAX_WAITS]
                keep = waits[len(extras):]
                for i in range(0, len(extras), MAX_WAITS):
                    chunk = extras[i : i + MAX_WAITS]
                    nop = mybir.InstNoOp(
                        name=nc.get_next_instruction_name(), ins=[], outs=[]
                    )
                    nop.engine = ins.engine
                    nop.sync_info = mybir.SyncInfo(on_wait=chunk, on_update=[])
                    out.append(nop)
                ins.sync_info = mybir.SyncInfo(
                    on_wait=keep, on_update=list(si.on_update) if si.on_update else []
                )
            out.append(ins)
        if changed:
            bb.instructions = out
    return nc


def _build_program():
    import concourse.bass as bass
    import concourse.mybir as mybir
    import concourse.tile as tile
    from concourse.masks import make_identity

    _install_tile_drain_patch()

    f32 = mybir.dt.float32
    f32r = mybir.dt.float32r
    fp16 = mybir.dt.float16

    nc = bass.Bass("TRN2", target_bir_lowering=False, debug=False)

    # DRAM I/O (per core).  x is laid out [KE, B, 128, S] so each
    # [128, S] projection tile is one fully contiguous 512KB DMA read.
    xq = nc.dram_tensor("xq", [KE, B, 128, S], fp16, kind="ExternalInput").ap()
    xk = nc.dram_tensor("xk", [KE, B, 128, S], fp16, kind="ExternalInput").ap()
    xv = nc.dram_tensor("xv", [KE, B, 128, S], fp16, kind="ExternalInput").ap()
    wq = nc.dram_tensor("wq", [128, KE, DOUT], fp16, kind="ExternalInput").ap()
    wk = nc.dram_tensor("wk", [128, KE, DOUT], fp16, kind="ExternalInput").ap()
    wv = nc.dram_tensor("wv", [128, KE, DOUT], fp16, kind="ExternalInput").ap()
    wo = nc.dram_tensor("wo", [DOUT, E], fp16, kind="ExternalInput").ap()
    bq = nc.dram_tensor("bq", [DOUT, 1], f32, kind="ExternalInput").ap()
    bk = nc.dram_tensor("bk", [DOUT, 1], f32, kind="ExternalInput").ap()
    bv = nc.dram_tensor("bv", [DOUT, 1], f32, kind="ExternalInput").ap()
    out = nc.dram_tensor("out", [NSEQ, E], fp16, kind="ExternalOutput").ap()

    with tile.TileContext(nc) as tc:
        with (
            nc.allow_low_precision(reason="fp16 attention pipeline"),
            tc.tile_pool(name="consts", bufs=1) as consts,
            tc.tile_pool(name="persist", bufs=1) as persist,
            tc.tile_pool(name="xstream", bufs=12) as xstream,
            tc.tile_pool(name="ptp", bufs=6) as ptp,
            tc.tile_pool(name="outp", bufs=3) as outp,
            tc.tile_pool(name="small", bufs=10) as small,
            tc.tile_pool(name="pp_ps", bufs=2, space="PSUM") as pp_ps,
            tc.tile_pool(name="sc_ps", bufs=2, space="PSUM") as sc_ps,
            tc.tile_pool(name="cx_ps", bufs=2, space="PSUM") as cx_ps,
        ):
            # ---- constants / persistent SBUF state ----
            ident_f32 = consts.tile([128, 128], f32)
            make_identity(nc, ident_f32[:])
            ident = consts.tile([128, 128], fp16)
            nc.vector.tensor_copy(ident[:], ident_f32[:])
            onesf = consts.tile([128, 1], f32)
            nc.vector.memset(onesf[:], 1.0)
            ones64h = consts.tile([1, 64], fp16)
            nc.vector.memset(ones64h[:], 1.0)
            neg8 = consts.tile([128, 1], f32)
            nc.vector.memset(neg8[:], -8.0)

            w_sb = {}
            b_sb = {}
            _wdram = {"q": (wq, bq), "k": (wk, bk), "v": (wv, bv)}

            def load_weights(name, eng=None):
                eng = eng or nc.sync
                wdram, bdram = _wdram[name]
                wt = persist.tile([128, KE, DOUT], fp16, tag=f"w{name}")
                eng.dma_start(wt[:], wdram[:])
                w_sb[name] = wt
                bt = persist.tile([DOUT, 1], f32, tag=f"b{name}")
                eng.dma_start(bt[:], bdram[:])
                b_sb[name] = bt

            wo_sb = persist.tile([DOUT, E], fp16, tag="wo")

            qt_sb = persist.tile([128, NSEQ], fp16, tag="qt")
            kt_sb = persist.tile([128, NSEQ], fp16, tag="kt")
            vt_sb = persist.tile([128, NSEQ], fp16, tag="vt")
            xT_sb = {"q": qt_sb, "k": kt_sb, "v": vt_sb}
            # [V | ones] per (kpos chunk, head): [128, 32, 2, 65] fp16
            v_sb = persist.tile([128, NSEQ // 128, 2, D + 1], fp16, tag="vn")
            nc.vector.tensor_copy(
                v_sb[:, :, :, D], onesf[:, 0:1].broadcast_to([128, NSEQ // 128, 2])
            )
            ctxT_sb = persist.tile([128, NSEQ], fp16, tag="ctxT")

            xdram = {"q": xq, "k": xk, "v": xv}

            # ---------------- step generators ----------------

            def proj_dma(b, name, eng=None, split=False):
                """Issue the 8 x-tile DMAs for (batch, tensor); returns tiles.
                split=True alternates tiles across both trigger queues so the
                stream lands in half the time (phase 0 only, while ACT idles)."""
                eng = eng or nc.sync
                tiles = []
                for k in range(KE):
                    xt = xstream.tile([128, S], fp16, tag="xs", name=f"x{b}{name}{k}")
                    e = (nc.sync, nc.scalar)[k % 2] if split else eng
                    e.dma_start(xt[:], xdram[name][k, b, :, :])
                    tiles.append(xt)
                return tiles

            def proj_chunk(b, name, tiles, sl):
                """One 512-wide projection slice: 8 accumulating matmuls."""
                ps = pp_ps.tile([128, SEQT], f32, tag="pp", name=f"pp{b}{name}{sl}")
                for k in range(KE):
                    nc.tensor.matmul(
                        ps[:],
                        lhsT=w_sb[name][:, k, :],
                        rhs=tiles[k][:, sl * SEQT : (sl + 1) * SEQT],
                        start=(k == 0),
                        stop=(k == KE - 1),
                    )
                nc.vector.tensor_scalar_add(
                    xT_sb[name][:, b * S + sl * SEQT : b * S + (sl + 1) * SEQT],
                    ps[:],
                    b_sb[name][:, 0:1],
                )

            def vtrans(ci):
                """Transpose one 128-col block of V^T into [V | ones] chunks."""
                tp = pp_ps.tile([128, 128], fp16, tag="pp", name=f"tp{ci}")
                nc.tensor.transpose(tp[:], vt_sb[:, ci * 128 : (ci + 1) * 128], ident[:])
                for h in range(2):
                    nc.vector.tensor_copy(
                        v_sb[:, ci, h, 0:D], tp[:, h * D : (h + 1) * D]
                    )

            def outproj_step(m, pool=None, tag="pp", split_drain=False):
                # PSUM->SBUF drains go to DVE; when ACT is idle (phase 3)
                # they are split across DVE and ACT.
                pool = pool or pp_ps
                ob = outp.tile([128, E], fp16, tag="ob", name=f"ob{m}")
                for n in range(2):
                    ps = pool.tile([128, SEQT], f32, tag=tag, name=f"op{m}{n}")
                    nc.tensor.matmul(
                        ps[:],
                        lhsT=ctxT_sb[:, m * 128 : (m + 1) * 128],
                        rhs=wo_sb[:, n * SEQT : (n + 1) * SEQT],
                        start=True,
                        stop=True,
                    )
                    osl = ob[:, n * SEQT : (n + 1) * SEQT]
                    if split_drain and n == 1:
                        nc.scalar.activation(
                            osl, ps[:], mybir.ActivationFunctionType.Copy
                        )
                    else:
                        nc.vector.tensor_copy(osl, ps[:])
                nc.sync.dma_start(out[m * 128 : (m + 1) * 128, :], ob[:])

            # Attention over one (batch, q-block): generator yielding once per
            # t-pair step so filler work can be interleaved by the driver.
            # Software-pipelined 2 deep: attnV of step tp is emitted during
            # tp+2, so the qb-boundary normalize latency (DVE reciprocal) is
            # covered by two steps of scores work.
            def attn_steps(b, qb):
                qsl = bass.ds(b * S + qb * SEQT, SEQT)
                ctx = [
                    cx_ps.tile([D + 1, SEQT], f32, tag="cx", name=f"cx{b}{qb}{h}")
                    for h in range(2)
                ]
                pend = []  # [(pt tiles, tp)] awaiting attnV emission
                for tp in range(KT // 2):
                    pts = []
                    for h in range(2):
                        hsl = bass.ts(h, D)
                        sct = sc_ps.tile(
                            [128, 2 * SEQT], f32, tag="sc", name=f"sc{b}{qb}{tp}{h}"
                        )
                        for j in range(2):
                            t = 2 * tp + j
                            ksl = bass.ds(b * S + t * 128, 128)
                            nc.tensor.matmul(
                                sct[:, j * SEQT : (j + 1) * SEQT],
                                lhsT=kt_sb[hsl, ksl],
                                rhs=qt_sb[hsl, qsl],
                                start=True,
                                stop=True,
                            )
                        ptt = ptp.tile(
                            [128, 2 * SEQT], fp16, tag="pt", name=f"pt{b}{qb}{tp}{h}"
                        )
                        # bias keeps exp within fp16 range (softmax is shift
                        # invariant; the denominator absorbs e^-8 exactly)
                        nc.scalar.activation(
                            ptt[:], sct[:], mybir.ActivationFunctionType.Exp,
                            scale=ISD, bias=neg8[:, 0:1],
                        )
                        pts.append(ptt)
                    pend.append((pts, tp))
                    if len(pend) > 2:
                        _emit_attnv(b, ctx, *pend.pop(0))
                    yield
                # drain the pipelined tail one step per slot, so fillers give
                # ACT time to finish the last exps and the PE wait queue
                # (depth 4) never holds more than one blocked group
                for p in pend:
                    yield
                    _emit_attnv(b, ctx, *p)
                # normalization part A: drain the ctx PSUM tiles with cheap
                # ops (denominator rows via ACT, context rows via DVE) so the
                # pool frees immediately, then start the reciprocals on the
                # copies, split across engines: h1 on ACT via
                # 1/x = exp(-ln x) (same act table as the scores exp), h0 on
                # the DVE iterative reciprocal — the two chains overlap.
                lns, recs = [], []
                for h in range(2):
                    lnt = small.tile([1, SEQT], f32, tag="dcp", name=f"ln{b}{qb}{h}")
                    nc.scalar.activation(
                        lnt[:], ctx[h][D : D + 1, :], mybir.ActivationFunctionType.Ln
                    )
                    lns.append(lnt)
                ctmps = []
                for h in range(2):
                    ctmp = small.tile([D, SEQT], f32, tag="ctmp", name=f"ct{b}{qb}{h}")
                    nc.vector.tensor_copy(ctmp[:], ctx[h][0:D, :])
                    ctmps.append(ctmp)
                for h in range(2):
                    rec = small.tile([1, SEQT], fp16, tag="rec", name=f"rc{b}{qb}{h}")
                    nc.scalar.activation(
                        rec[:], lns[h][:], mybir.ActivationFunctionType.Exp, scale=-1.0
                    )
                    recs.append(rec)
                yield
                yield
                # part B (two slots later, so the PE never waits on the
                # reciprocals): replicate over the 64 head dims + multiply.
                for h, rec_t in ((1, recs[1]), (0, recs[0])):
                    hsl = bass.ts(h, D)
                    rrep = pp_ps.tile([D, SEQT], f32, tag="pp", name=f"rp{b}{qb}{h}")
                    nc.tensor.matmul(
                        rrep[:], lhsT=ones64h[:], rhs=rec_t[:], start=True, stop=True
                    )
                    nc.vector.tensor_tensor(
                        out=ctxT_sb[hsl, qsl],
                        in0=ctmps[h][:],
                        in1=rrep[:],
                        op=mybir.AluOpType.mult,
                    )

            def _emit_attnv(b, ctx, pts, tp):
                for h in range(2):
                    for j in range(2):
                        t = 2 * tp + j
                        nc.tensor.matmul(
                            ctx[h][:],
                            lhsT=v_sb[:, b * KT + t, h, :],
                            rhs=pts[h][:, j * SEQT : (j + 1) * SEQT],
                            start=(t == 0),
                            stop=(t == KT - 1),
                        )

            # ---------------- emission schedule ----------------

            def run_interleaved(attn_gens, fillers, total_slots):
                """Drain attention generators, spreading filler closures
                evenly over the whole phase.  Fillers are emitted at slot
                starts so filler work always precedes the attention steps
                that may depend on it."""
                fi = 0
                slot = 0
                n = len(fillers)
                for g in attn_gens:
                    it = iter(g)
                    while True:
                        while fi < n and fi * total_slots <= slot * n:
                            fillers[fi]()
                            fi += 1
                        slot += 1
                        try:
                            next(it)
                        except StopIteration:
                            break
                while fi < n:
                    fillers[fi]()
                    fi += 1

            # Phase 0: batch-0 projections (k, q first; v interleaves with
            # the first attention block since scores only need K^T/Q^T).
            # Two trigger queues in parallel: sync carries k's weights+x
            # (the critical path), the idle ACT queue carries q's and v's.
            load_weights("k")
            load_weights("q", eng=nc.scalar)
            tiles_k = proj_dma(0, "k", split=True)
            tiles_q = proj_dma(0, "q", split=True)
            load_weights("v", eng=nc.scalar)
            nc.scalar.dma_start(wo_sb[:], wo[:])
            for sl in range(QB):
                proj_chunk(0, "k", tiles_k, sl)
            for sl in range(QB):
                proj_chunk(0, "q", tiles_q, sl)
            tiles_v0 = proj_dma(0, "v", split=True)

            # Phase 1: batch-0 attention with fillers = [v(b0) proj+transposes,
            # then all of batch-1 projections].
            fillers1 = []
            for sl in range(QB):
                fillers1.append(
                    lambda sl=sl: (
                        proj_chunk(0, "v", tiles_v0, sl),
                        [vtrans(ci) for ci in range(sl * 4, sl * 4 + 4)],
                    )
                )
            tiles_b1 = {}

            def dma_b1(name):
                tiles_b1[name] = proj_dma(1, name)

            fillers1.append(lambda: dma_b1("k"))
            for sl in range(QB):
                fillers1.append(lambda sl=sl: proj_chunk(1, "k", tiles_b1["k"], sl))
            fillers1.append(lambda: dma_b1("q"))
            for sl in range(QB):
                fillers1.append(lambda sl=sl: proj_chunk(1, "q", tiles_b1["q"], sl))
            fillers1.append(lambda: dma_b1("v"))
            for sl in range(QB):
                fillers1.append(
                    lambda sl=sl: (
                        proj_chunk(1, "v", tiles_b1["v"], sl),
                        [vtrans(ci) for ci in range(16 + sl * 4, 16 + sl * 4 + 4)],
                    )
                )
            run_interleaved([attn_steps(0, qb) for qb in range(QB)], fillers1, 48)

            # Phase 2: batch-1 attention with out-projection fillers — all of
            # batch 0 plus the first half of batch 1 (whose ctxT normalizes
            # early enough in this same phase).
            # out-projections back-loaded: the early phase-2 slots are already
            # PE-saturated; the famine is at the end of the phase.
            fillers2 = [lambda: None] * 12 + [
                lambda m=m: outproj_step(m) for m in range(24)
            ]
            run_interleaved([attn_steps(1, qb) for qb in range(QB)], fillers2, 48)

            # Phase 3: out-projection tail (alternate PSUM pools so four
            # banks hide the drain latency).
            for m in range(24, 32):
                if m % 2:
                    outproj_step(m, pool=sc_ps, tag="sc", split_drain=True)
                else:
                    outproj_step(m, split_drain=True)

    return nc


def _get_program():
    global _PROGRAM
    if _PROGRAM is None:
        _PROGRAM = _split_excess_waits(_build_program())
    return _PROGRAM


def kernel(query, key, value, Wq, bq, Wk, bk, Wv, bv, Wo, bo):
    from concourse.bass_utils import run_bass_kernel_spmd

    nc = _get_program()

    fp16 = np.float16
    q2 = np.asarray(query, np.float32).reshape(NSEQ, E)
    k2 = np.asarray(key, np.float32).reshape(NSEQ, E)
    v2 = np.asarray(value, np.float32).reshape(NSEQ, E)
    # x^T [E, NSEQ] -> [KE, B, 128, S] (each [128, S] tile contiguous),
    # rounded to fp16 on host (the fp16 matmul rounds its inputs anyway)
    def xprep(x2):
        xt = x2.T.reshape(KE, 128, B, S).transpose(0, 2, 1, 3)
        return np.ascontiguousarray(xt).astype(fp16)

    xq = xprep(q2)
    xk = xprep(k2)
    xv = xprep(v2)

    Wq = np.asarray(Wq, np.float32)
    Wk = np.asarray(Wk, np.float32)
    Wv = np.asarray(Wv, np.float32)
    Wo = np.asarray(Wo, np.float32)

    def wprep(W, rsl):
        # lhsT for the projections, laid out [partition, k, dout]
        return np.ascontiguousarray(
            W[rsl, :].T.reshape(KE, 128, DOUT).transpose(1, 0, 2)
        ).astype(fp16)

    in_maps = []
    for c in range(NCORES):
        rsl = slice(DOUT * c, DOUT * (c + 1))
        in_maps.append(
            {
                "xq": xq, "xk": xk, "xv": xv,
                "wq": wprep(Wq, rsl),
                "wk": wprep(Wk, rsl),
                "wv": wprep(Wv, rsl),
                # rhs for the out-proj: rows c-range of Wo^T  [DOUT, E]
                "wo": np.ascontiguousarray(Wo[:, rsl].T).astype(fp16),
                "bq": np.ascontiguousarray(np.asarray(bq, np.float32)[rsl]).reshape(DOUT, 1),
                "bk": np.ascontiguousarray(np.asarray(bk, np.float32)[rsl]).reshape(DOUT, 1),
                "bv": np.ascontiguousarray(np.asarray(bv, np.float32)[rsl]).reshape(DOUT, 1),
            }
        )

    res = run_bass_kernel_spmd(nc, in_maps, list(range(NCORES)), trace=False)
    acc = np.zeros((NSEQ, E), np.float32)
    for c in range(NCORES):
        acc += res.results[c]["out"].astype(np.float32)
    acc += np.asarray(bo, np.float32)[None, :]
    return acc.reshape(B, S, E)


# revision 51
# speedup vs baseline: 1.0245x; 1.0245x over previous
"""Multihead attention (B=2, S=2048, E=1024, H=16) on 8 TRN2 cores.

Sharding: tensor-parallel over heads — core c computes heads {2c, 2c+1}
(DOUT = 128 columns of the QKV projections) for the full sequence, then its
partial contribution to the output projection; the host sums the 8 partials
and adds the output bias.

v2 layout (per core), all 2-byte tensors in fp16 (same PE rate as bf16,
~8x lower rounding error):
  x^T [E, B*S] fp16 streamed as [128, 2048] tiles (4KB DMA lines, full HBM
  rate).  QKV projections produce Q^T/K^T/V^T [128, 4096] fp16 in SBUF.
  Attention per (batch, head) computes scores^T [kpos, q] tiles (lhsT = K^T
  slice, rhs = Q^T slice), exponentiates on ACT over [128, 1024] PSUM tiles,
  and multiplies by V via matmul with lhsT = [V | ones] so the softmax
  denominator falls out of the same accumulation (PSUM row 64).  The
  reciprocal runs on ACT (DVE reciprocal on a 1-partition tile is ~4us),
  is broadcast over 64 partitions by a tiny PE matmul, and the normalized
  context lands in ctxT [128, 4096] fp16.  The output projection contracts
  the 128 local head dims in fp16 and streams out partial sums as fp16.

Emission is software-pipelined so the PE never stalls on ACT: per t-pair
step the attnV matmuls of the PREVIOUS step are emitted after the scores
of the current one, and projection / out-projection chunks of the other
batch are interleaved between attention steps as PE gap filler.
"""

import numpy as np

# Problem constants (hardcoded per the task contract).
B, S, E, H = 2, 2048, 1024, 16
D = E // H          # 64
NSEQ = B * S        # 4096
NCORES = 8
DOUT = E // NCORES  # 128 = 2 heads x 64
KE = E // 128       # 8 contraction tiles over E
SEQT = 512          # q-block for attention
QB = S // SEQT      # 4 q-blocks per batch
KT = S // 128       # 16 kpos tiles per batch
ISD = float(D) ** -0.5

_PROGRAM = None


# ---------------------------------------------------------------------------
# Workarounds for this walrus build: at most ONE sync wait per instruction is
# reliably accepted ("Too many sync wait commands").  (1) tile's final drain
# gets one wait per logical proc — split them over single-wait SP NOPs;
# (2) a general post-pass moves any instruction's excess waits onto
# preceding same-engine NOPs (engine program order preserves semantics).
# ---------------------------------------------------------------------------


def _install_tile_drain_patch():
    import concourse.mybir as mybir
    import concourse.tile as tile
    from concourse.tile import ScopedClock

    if getattr(tile.TileContext, "_drain_patch_installed", False):
        return

    def _patched_drain_and_barrier(self, tick_clock, wait_clock):
        nc = self.nc
        carrier = nc.sync.nop(nofuse=True)
        wait_clock.add_sem_waits(
            carrier.ins, ScopedClock({None: tick_clock.global_clock})
        )
        si = carrier.ins.sync_info
        waits = list(si.on_wait) if si and si.on_wait else []
        ups = list(si.on_update) if si and si.on_update else []
        if len(waits) > 1:
            carrier.ins.sync_info = mybir.SyncInfo(on_wait=[waits[0]], on_update=ups)
            for w in waits[1:]:
                n2 = nc.sync.nop(nofuse=True)
                n2.ins.sync_info = mybir.SyncInfo(on_wait=[w], on_update=[])
        nc.sync.drain()
        nc.all_engine_barrier()
        popped = nc._tile_sem_poison_stack.pop()
        assert popped is self._sem_poison
        nc.clear_and_free_semaphores(list(self.sems.allocated().values()))
        nc.all_engine_barrier()

    tile.TileContext._drain_and_barrier = _patched_drain_and_barrier
    tile.TileContext._drain_patch_installed = True


MAX_WAITS = 1


def _split_excess_waits(nc):
    import concourse.mybir as mybir

    for bb in nc.main_func.blocks:
        il = list(bb.instructions)
        out = []
        changed = False
        for ins in il:
            si = ins.sync_info
            waits = list(si.on_wait) if si and si.on_wait else []
            if len(waits) > MAX_WAITS:
                changed = True
                extras = waits[: len(waits) - MAX_WAITS]
                keep = waits[len(extras):]
                for i in range(0, len(extras), MAX_WAITS):
                    chunk = extras[i : i + MAX_WAITS]
                    nop = mybir.InstNoOp(
                        name=nc.get_next_instruction_name(), ins=[], outs=[]
                    )
                    nop.engine = ins.engine
                    nop.sync_info = mybir.SyncInfo(on_wait=chunk, on_update=[])
                    out.append(nop)
                ins.sync_info = mybir.SyncInfo(
                    on_wait=keep, on_update=list(si.on_update) if si.on_update else []
                )
            out.append(ins)
        if changed:
            bb.instructions = out
    return nc


def _build_program():
    import concourse.bass as bass
    import concourse.mybir as mybir
    import concourse.tile as tile
    from concourse.masks import make_identity

    _install_tile_drain_patch()

    f32 = mybir.dt.float32
    f32r = mybir.dt.float32r
    fp16 = mybir.dt.float16

    nc = bass.Bass("TRN2", target_bir_lowering=False, debug=False)

    # DRAM I/O (per core).  x is laid out [KE, B, 128, S] so each
    # [128, S] projection tile is one fully contiguous 512KB DMA read.
    xq = nc.dram_tensor("xq", [KE, B, 128, S], fp16, kind="ExternalInput").ap()
    xk = nc.dram_tensor("xk", [KE, B, 128, S], fp16, kind="ExternalInput").ap()
    xv = nc.dram_tensor("xv", [KE, B, 128, S], fp16, kind="ExternalInput").ap()
    wq = nc.dram_tensor("wq", [128, KE, DOUT], fp16, kind="ExternalInput").ap()
    wk = nc.dram_tensor("wk", [128, KE, DOUT], fp16, kind="ExternalInput").ap()
    wv = nc.dram_tensor("wv", [128, KE, DOUT], fp16, kind="ExternalInput").ap()
    wo = nc.dram_tensor("wo", [DOUT, E], fp16, kind="ExternalInput").ap()
    bq = nc.dram_tensor("bq", [DOUT, 1], f32, kind="ExternalInput").ap()
    bk = nc.dram_tensor("bk", [DOUT, 1], f32, kind="ExternalInput").ap()
    bv = nc.dram_tensor("bv", [DOUT, 1], f32, kind="ExternalInput").ap()
    out = nc.dram_tensor("out", [NSEQ, E], fp16, kind="ExternalOutput").ap()

    with tile.TileContext(nc) as tc:
        with (
            nc.allow_low_precision(reason="fp16 attention pipeline"),
            tc.tile_pool(name="consts", bufs=1) as consts,
            tc.tile_pool(name="persist", bufs=1) as persist,
            tc.tile_pool(name="xstream", bufs=12) as xstream,
            tc.tile_pool(name="ptp", bufs=6) as ptp,
            tc.tile_pool(name="outp", bufs=3) as outp,
            tc.tile_pool(name="small", bufs=10) as small,
            tc.tile_pool(name="pp_ps", bufs=2, space="PSUM") as pp_ps,
            tc.tile_pool(name="sc_ps", bufs=2, space="PSUM") as sc_ps,
            tc.tile_pool(name="cx_ps", bufs=2, space="PSUM") as cx_ps,
        ):
            # ---- constants / persistent SBUF state ----
            ident_f32 = consts.tile([128, 128], f32)
            make_identity(nc, ident_f32[:])
            ident = consts.tile([128, 128], fp16)
            nc.vector.tensor_copy(ident[:], ident_f32[:])
            onesf = consts.tile([128, 1], f32)
            nc.vector.memset(onesf[:], 1.0)
            ones64h = consts.tile([1, 64], fp16)
            nc.vector.memset(ones64h[:], 1.0)
            neg8 = consts.tile([128, 1], f32)
            nc.vector.memset(neg8[:], -8.0)

            w_sb = {}
            b_sb = {}
            _wdram = {"q": (wq, bq), "k": (wk, bk), "v": (wv, bv)}

            def load_weights(name, eng=None):
                eng = eng or nc.sync
                wdram, bdram = _wdram[name]
                wt = persist.tile([128, KE, DOUT], fp16, tag=f"w{name}")
                eng.dma_start(wt[:], wdram[:])
                w_sb[name] = wt
                bt = persist.tile([DOUT, 1], f32, tag=f"b{name}")
                eng.dma_start(bt[:], bdram[:])
                b_sb[name] = bt

            wo_sb = persist.tile([DOUT, E], fp16, tag="wo")

            qt_sb = persist.tile([128, NSEQ], fp16, tag="qt")
            kt_sb = persist.tile([128, NSEQ], fp16, tag="kt")
            vt_sb = persist.tile([128, NSEQ], fp16, tag="vt")
            xT_sb = {"q": qt_sb, "k": kt_sb, "v": vt_sb}
            # [V | ones] per (kpos chunk, head): [128, 32, 2, 65] fp16
            v_sb = persist.tile([128, NSEQ // 128, 2, D + 1], fp16, tag="vn")
            nc.vector.tensor_copy(
                v_sb[:, :, :, D], onesf[:, 0:1].broadcast_to([128, NSEQ // 128, 2])
            )
            ctxT_sb = persist.tile([128, NSEQ], fp16, tag="ctxT")

            xdram = {"q": xq, "k": xk, "v": xv}

            # ---------------- step generators ----------------

            def proj_dma(b, name, eng=None):
                """Issue the 8 x-tile DMAs for (batch, tensor); returns tiles."""
                eng = eng or nc.sync
                tiles = []
                for k in range(KE):
                    xt = xstream.tile([128, S], fp16, tag="xs", name=f"x{b}{name}{k}")
                    eng.dma_start(xt[:], xdram[name][k, b, :, :])
                    tiles.append(xt)
                return tiles

            def proj_chunk(b, name, tiles, sl):
                """One 512-wide projection slice: 8 accumulating matmuls."""
                ps = pp_ps.tile([128, SEQT], f32, tag="pp", name=f"pp{b}{name}{sl}")
                for k in range(KE):
                    nc.tensor.matmul(
                        ps[:],
                        lhsT=w_sb[name][:, k, :],
                        rhs=tiles[k][:, sl * SEQT : (sl + 1) * SEQT],
                        start=(k == 0),
                        stop=(k == KE - 1),
                    )
                nc.vector.tensor_scalar_add(
                    xT_sb[name][:, b * S + sl * SEQT : b * S + (sl + 1) * SEQT],
                    ps[:],
                    b_sb[name][:, 0:1],
                )

            def vtrans(ci):
                """Transpose one 128-col block of V^T into [V | ones] chunks."""
                tp = pp_ps.tile([128, 128], fp16, tag="pp", name=f"tp{ci}")
                nc.tensor.transpose(tp[:], vt_sb[:, ci * 128 : (ci + 1) * 128], ident[:])
                for h in range(2):
                    nc.vector.tensor_copy(
                        v_sb[:, ci, h, 0:D], tp[:, h * D : (h + 1) * D]
                    )

            def outproj_step(m, pool=None, tag="pp", split_drain=False):
                # PSUM->SBUF drains go to DVE; when ACT is idle (phase 3)
                # they are split across DVE and ACT.
                pool = pool or pp_ps
                ob = outp.tile([128, E], fp16, tag="ob", name=f"ob{m}")
                for n in range(2):
                    ps = pool.tile([128, SEQT], f32, tag=tag, name=f"op{m}{n}")
                    nc.tensor.matmul(
                        ps[:],
                        lhsT=ctxT_sb[:, m * 128 : (m + 1) * 128],
                        rhs=wo_sb[:, n * SEQT : (n + 1) * SEQT],
                        start=True,
                        stop=True,
                    )
                    osl = ob[:, n * SEQT : (n + 1) * SEQT]
                    if split_drain and n == 1:
                        nc.scalar.activation(
                            osl, ps[:], mybir.ActivationFunctionType.Copy
                        )
                    else:
                        nc.vector.tensor_copy(osl, ps[:])
                nc.sync.dma_start(out[m * 128 : (m + 1) * 128, :], ob[:])

            # Attention over one (batch, q-block): generator yielding once per
            # t-pair step so filler work can be interleaved by the driver.
            # Software-pipelined 2 deep: attnV of step tp is emitted during
            # tp+2, so the qb-boundary normalize latency (DVE reciprocal) is
            # covered by two steps of scores work.
            def attn_steps(b, qb):
                qsl = bass.ds(b * S + qb * SEQT, SEQT)
                ctx = [
                    cx_ps.tile([D + 1, SEQT], f32, tag="cx", name=f"cx{b}{qb}{h}")
                    for h in range(2)
                ]
                pend = []  # [(pt tiles, tp)] awaiting attnV emission
                for tp in range(KT // 2):
                    pts = []
                    for h in range(2):
                        hsl = bass.ts(h, D)
                        sct = sc_ps.tile(
                            [128, 2 * SEQT], f32, tag="sc", name=f"sc{b}{qb}{tp}{h}"
                        )
                        for j in range(2):
                            t = 2 * tp + j
                            ksl = bass.ds(b * S + t * 128, 128)
                            nc.tensor.matmul(
                                sct[:, j * SEQT : (j + 1) * SEQT],
                                lhsT=kt_sb[hsl, ksl],
                                rhs=qt_sb[hsl, qsl],
                                start=True,
                                stop=True,
                            )
                        ptt = ptp.tile(
                            [128, 2 * SEQT], fp16, tag="pt", name=f"pt{b}{qb}{tp}{h}"
                        )
                        # bias keeps exp within fp16 range (softmax is shift
                        # invariant; the denominator absorbs e^-8 exactly)
                        nc.scalar.activation(
                            ptt[:], sct[:], mybir.ActivationFunctionType.Exp,
                            scale=ISD, bias=neg8[:, 0:1],
                        )
                        pts.append(ptt)
                    pend.append((pts, tp))
                    if len(pend) > 2:
                        _emit_attnv(b, ctx, *pend.pop(0))
                    yield
                # drain the pipelined tail one step per slot, so fillers give
                # ACT time to finish the last exps and the PE wait queue
                # (depth 4) never holds more than one blocked group
                for p in pend:
                    yield
                    _emit_attnv(b, ctx, *p)
                # normalization part A: drain the ctx PSUM tiles with cheap
                # ops (denominator rows via ACT, context rows via DVE) so the
                # pool frees immediately, then start the reciprocals on the
                # copies, split across engines: h1 on ACT via
                # 1/x = exp(-ln x) (same act table as the scores exp), h0 on
                # the DVE iterative reciprocal — the two chains overlap.
                lns, recs = [], []
                for h in range(2):
                    lnt = small.tile([1, SEQT], f32, tag="dcp", name=f"ln{b}{qb}{h}")
                    nc.scalar.activation(
                        lnt[:], ctx[h][D : D + 1, :], mybir.ActivationFunctionType.Ln
                    )
                    lns.append(lnt)
                ctmps = []
                for h in range(2):
                    ctmp = small.tile([D, SEQT], f32, tag="ctmp", name=f"ct{b}{qb}{h}")
                    nc.vector.tensor_copy(ctmp[:], ctx[h][0:D, :])
                    ctmps.append(ctmp)
                for h in range(2):
                    rec = small.tile([1, SEQT], fp16, tag="rec", name=f"rc{b}{qb}{h}")
                    nc.scalar.activation(
                        rec[:], lns[h][:], mybir.ActivationFunctionType.Exp, scale=-1.0
                    )
                    recs.append(rec)
                yield
                yield
                # part B (two slots later, so the PE never waits on the
                # reciprocals): replicate over the 64 head dims + multiply.
                for h, rec_t in ((1, recs[1]), (0, recs[0])):
                    hsl = bass.ts(h, D)
                    rrep = pp_ps.tile([D, SEQT], f32, tag="pp", name=f"rp{b}{qb}{h}")
                    nc.tensor.matmul(
                        rrep[:], lhsT=ones64h[:], rhs=rec_t[:], start=True, stop=True
                    )
                    nc.vector.tensor_tensor(
                        out=ctxT_sb[hsl, qsl],
                        in0=ctmps[h][:],
                        in1=rrep[:],
                        op=mybir.AluOpType.mult,
                    )

            def _emit_attnv(b, ctx, pts, tp):
                for h in range(2):
                    for j in range(2):
                        t = 2 * tp + j
                        nc.tensor.matmul(
                            ctx[h][:],
                            lhsT=v_sb[:, b * KT + t, h, :],
                            rhs=pts[h][:, j * SEQT : (j + 1) * SEQT],
                            start=(t == 0),
                            stop=(t == KT - 1),
                        )

            # ---------------- emission schedule ----------------

            def run_interleaved(attn_gens, fillers, total_slots):
                """Drain attention generators, spreading filler closures
                evenly over the whole phase.  Fillers are emitted at slot
                starts so filler work always precedes the attention steps
                that may depend on it."""
                fi = 0
                slot = 0
                n = len(fillers)
                for g in attn_gens:
                    it = iter(g)
                    while True:
                        while fi < n and fi * total_slots <= slot * n:
                            fillers[fi]()
                            fi += 1
                        slot += 1
                        try:
                            next(it)
                        except StopIteration:
                            break
                while fi < n:
                    fillers[fi]()
                    fi += 1

            # Phase 0: batch-0 projections (k, q first; v interleaves with
            # the first attention block since scores only need K^T/Q^T).
            # Two trigger queues in parallel: sync carries k's weights+x
            # (the critical path), the idle ACT queue carries q's and v's.
            load_weights("k")
            load_weights("q", eng=nc.scalar)
            tiles_k = proj_dma(0, "k")
            tiles_q = proj_dma(0, "q", eng=nc.scalar)
            load_weights("v", eng=nc.scalar)
            nc.scalar.dma_start(wo_sb[:], wo[:])
            for sl in range(QB):
                proj_chunk(0, "k", tiles_k, sl)
            for sl in range(QB):
                proj_chunk(0, "q", tiles_q, sl)
            tiles_v0 = proj_dma(0, "v", eng=nc.scalar)

            # Phase 1: batch-0 attention with fillers = [v(b0) proj+transposes,
            # then all of batch-1 projections].
            fillers1 = []
            for sl in range(QB):
                fillers1.append(
                    lambda sl=sl: (
                        proj_chunk(0, "v", tiles_v0, sl),
                        [vtrans(ci) for ci in range(sl * 4, sl * 4 + 4)],
                    )
                )
            tiles_b1 = {}

            def dma_b1(name):
                tiles_b1[name] = proj_dma(1, name)

            fillers1.append(lambda: dma_b1("k"))
            for sl in range(QB):
                fillers1.append(lambda sl=sl: proj_chunk(1, "k", tiles_b1["k"], sl))
            fillers1.append(lambda: dma_b1("q"))
            for sl in range(QB):
                fillers1.append(lambda sl=sl: proj_chunk(1, "q", tiles_b1["q"], sl))
            fillers1.append(lambda: dma_b1("v"))
            for sl in range(QB):
                fillers1.append(
                    lambda sl=sl: (
                        proj_chunk(1, "v", tiles_b1["v"], sl),
                        [vtrans(ci) for ci in range(16 + sl * 4, 16 + sl * 4 + 4)],
                    )
                )
            run_interleaved([attn_steps(0, qb) for qb in range(QB)], fillers1, 48)

            # Phase 2: batch-1 attention with out-projection fillers — all of
            # batch 0 plus the first half of batch 1 (whose ctxT normalizes
            # early enough in this same phase).
            fillers2 = [lambda m=m: outproj_step(m) for m in range(24)]
            run_interleaved([attn_steps(1, qb) for qb in range(QB)], fillers2, 48)

            # Phase 3: out-projection tail (alternate PSUM pools so four
            # banks hide the drain latency).
            for m in range(24, 32):
                if m % 2:
                    outproj_step(m, pool=sc_ps, tag="sc", split_drain=True)
                else:
                    outproj_step(m, split_drain=True)

    return nc


def _get_program():
    global _PROGRAM
    if _PROGRAM is None:
        _PROGRAM = _split_excess_waits(_build_program())
    return _PROGRAM


def kernel(query, key, value, Wq, bq, Wk, bk, Wv, bv, Wo, bo):
    from concourse.bass_utils import run_bass_kernel_spmd

    nc = _get_program()

    fp16 = np.float16
    q2 = np.asarray(query, np.float32).reshape(NSEQ, E)
    k2 = np.asarray(key, np.float32).reshape(NSEQ, E)
    v2 = np.asarray(value, np.float32).reshape(NSEQ, E)
    # x^T [E, NSEQ] -> [KE, B, 128, S] (each [128, S] tile contiguous),
    # rounded to fp16 on host (the fp16 matmul rounds its inputs anyway)
    def xprep(x2):
        xt = x2.T.reshape(KE, 128, B, S).transpose(0, 2, 1, 3)
        return np.ascontiguousarray(xt).astype(fp16)

    xq = xprep(q2)
    xk = xprep(k2)
    xv = xprep(v2)

    Wq = np.asarray(Wq, np.float32)
    Wk = np.asarray(Wk, np.float32)
    Wv = np.asarray(Wv, np.float32)
    Wo = np.asarray(Wo, np.float32)

    def wprep(W, rsl):
        # lhsT for the projections, laid out [partition, k, dout]
        return np.ascontiguousarray(
            W[rsl, :].T.reshape(KE, 128, DOUT).transpose(1, 0, 2)
        ).astype(fp16)

    in_maps = []
    for c in range(NCORES):
        rsl = slice(DOUT * c, DOUT * (c + 1))
        in_maps.append(
            {
                "xq": xq, "xk": xk, "xv": xv,
                "wq": wprep(Wq, rsl),
                "wk": wprep(Wk, rsl),
                "wv": wprep(Wv, rsl),
                # rhs for the out-proj: rows c-range of Wo^T  [DOUT, E]
                "wo": np.ascontiguousarray(Wo[:, rsl].T).astype(fp16),
                "bq": np.ascontiguousarray(np.asarray(bq, np.float32)[rsl]).reshape(DOUT, 1),
                "bk": np.ascontiguousarray(np.asarray(bk, np.float32)[rsl]).reshape(DOUT, 1),
                "bv": np.ascontiguousarray(np.asarray(bv, np.float32)[rsl]).reshape(DOUT, 1),
            }
        )

    res = run_bass_kernel_spmd(nc, in_maps, list(range(NCORES)), trace=False)
    acc = np.zeros((NSEQ, E), np.float32)
    for c in range(NCORES):
        acc += res.results[c]["out"].astype(np.float32)
    acc += np.asarray(bo, np.float32)[None, :]
    return acc.reshape(B, S, E)
